# revision 10
# baseline (speedup 1.0000x reference)
"""Trainium2 Bass kernel for nn_MultiHeadHierarchicalAttention.

Sharding (8 cores): 4 batches x 2 head-groups of 4 heads.  Each core runs the
full hierarchical attention for its (batch, head-group) slice and emits a
partial [LQ, D] output (final projections folded: fcs_w@fco_top / fcw_w@fco_bot
precomputed on host).  Host gathers: out[b] = part[2b] + part[2b+1] + v0.

Device compute is bf16 (fp32 PSUM accumulation).  Softmax is unnormalized
exp (no max subtraction; scores are O(1)), with the row-sum Z obtained via a
ones-column appended to V in the PV matmul.  The word-attention sentence
renorm (out_w *= attns) and 1/Z are applied per-partition post-PV on VE.

Self-contained: needs only numpy + ml_dtypes + the installed concourse pkg.
"""
import sys
import numpy as np
import ml_dtypes
from contextlib import ExitStack

try:
    import concourse.bass as bass
except ImportError:  # fallback if concourse isn't on sys.path in fresh dirs
    sys.path.insert(0, "/opt/trn_rl_repo")
    import concourse.bass as bass
import concourse.tile as tile
from concourse import mybir

# ---- problem constants ----
B, LQ, NS, NT, D = 4, 256, 24, 128, 512
H, DK = 8, 64
H4 = 4            # heads per core
HD = H4 * DK      # 256
SCALE = DK ** -0.5
N_CORES = 8

BF16 = ml_dtypes.bfloat16
DT_BF = mybir.dt.bfloat16
DT_F32 = mybir.dt.float32
ALU = mybir.AluOpType
AF = mybir.ActivationFunctionType

UCOL = 65          # 64 d + 1 ones column
S_HALF = 12        # sentences per U half-buffer
S_PER_BANK = 6     # 6 * 65 = 390 <= 512 fp32 per bank


def build_core_program(nc: bass.Bass):
    dram = {}

    def din(name, shape, dtype=DT_BF):
        dram[name] = nc.dram_tensor(name, list(shape), dtype, kind="ExternalInput")
        return dram[name]

    qT = din("qT", (D, LQ))
    k_sT = din("k_sT", (D, NS))
    v_sT = din("v_sT", (D, NS))
    k_wT = din("k_wT", (D, NS * NT))
    v_wT = din("v_wT", (D, NS * NT))
    biasT = din("biasT", (NS, H4, NT, LQ))       # (s, h, t, q)
    bias_sq = din("bias_sq", (H4, LQ, NS))       # (h, q, s)
    Gt = din("G", (H4, NS, NS))
    wq_s = din("wq_s", (D, HD))                  # pre-scaled by DK**-0.5
    wk_s = din("wk_s", (D, HD))
    wv_s = din("wv_s", (D, HD))
    wq_w = din("wq_w", (D, HD))                  # pre-scaled
    wk_w = din("wk_w", (D, HD))
    wv_w = din("wv_w", (D, HD))
    Ws_eff = din("Ws_eff", (HD, D))
    Ww_eff = din("Ww_eff", (HD, D))
    ident_bf = din("ident_bf", (128, 128), DT_BF)
    ident_f32 = din("ident_f32", (128, 128), DT_F32)
    pb_q_s = din("pb_q_s", (128, 2), DT_F32)
    pb_k_s = din("pb_k_s", (128, 2), DT_F32)
    pb_q_w = din("pb_q_w", (128, 2), DT_F32)
    pb_k_w = din("pb_k_w", (128, 2), DT_F32)
    rb_v_s = din("rb_v_s", (1, HD))
    rb_v_w = din("rb_v_w", (1, HD))

    out_t = nc.dram_tensor("out", [LQ, D], DT_F32, kind="ExternalOutput")

    with tile.TileContext(nc) as tc, ExitStack() as ctx:
        P = ctx.enter_context(tc.tile_pool(name="persist", bufs=1))

        def ptile(name, shape, dtype):
            return P.tile(shape, dtype, name=name, tag=name)

        identB = ptile("identB", [128, 128], DT_BF)
        nc.sync.dma_start(identB[:], ident_bf.ap())
        identF = ptile("identF", [128, 128], DT_F32)
        nc.sync.dma_start(identF[:], ident_f32.ap())
        ones_row = ptile("ones_row", [1, 128], DT_BF)
        nc.vector.memset(ones_row[:], 1.0)

        def load_w(name, ncols, rows=D):
            c = rows // 128
            t = ptile(f"sb_{name}", [128, c * ncols], DT_BF)
            nc.sync.dma_start(
                t[:].rearrange("p (c n) -> p c n", c=c),
                dram[name].ap().rearrange("(c p) n -> p c n", p=128))
            return t

        sb_wq_s = load_w("wq_s", HD)
        sb_wk_s = load_w("wk_s", HD)
        sb_wv_s = load_w("wv_s", HD)
        sb_wq_w = load_w("wq_w", HD)
        sb_wk_w = load_w("wk_w", HD)
        sb_wv_w = load_w("wv_w", HD)
        sb_Ws = load_w("Ws_eff", D, rows=HD)      # [128, 2*512]
        sb_Ww = load_w("Ww_eff", D, rows=HD)
        sb_qT = load_w("qT", LQ)                  # [128, 4*256]
        sb_k_sT = load_w("k_sT", NS)
        sb_v_sT = load_w("v_sT", NS)

        sb_pb = {}
        for nm in ("pb_q_s", "pb_k_s", "pb_q_w", "pb_k_w"):
            t = ptile(f"sb_{nm}", [128, 2], DT_F32)
            nc.sync.dma_start(t[:], dram[nm].ap())
            sb_pb[nm] = t
        sb_rb_v_s = ptile("sb_rb_v_s", [1, HD], DT_BF)
        nc.sync.dma_start(sb_rb_v_s[:], rb_v_s.ap())
        sb_rb_v_w = ptile("sb_rb_v_w", [1, HD], DT_BF)
        nc.sync.dma_start(sb_rb_v_w[:], rb_v_w.ap())
        sb_bias_sq = ptile("sb_bias_sq", [128, H4 * 2 * NS], DT_BF)
        nc.sync.dma_start(
            sb_bias_sq[:].rearrange("p (h c s) -> p h c s", h=H4, c=2),
            bias_sq.ap().rearrange("h (c p) s -> p h c s", p=128))
        sb_G = ptile("sb_G", [NS, H4 * NS], DT_BF)
        nc.sync.dma_start(sb_G[:].rearrange("s (h t) -> s h t", h=H4),
                          Gt.ap().rearrange("h s t -> s h t"))

        kwnT = ptile("kwnT", [128, 2 * NS * NT], DT_BF)    # [hd-c | st]
        vwone = ptile("vwone", [128, NS * H4 * UCOL], DT_BF)
        nc.vector.memset(vwone[:], 1.0)
        qwT = ptile("qwT", [128, 2 * LQ], DT_BF)
        qsT = ptile("qsT", [128, 2 * LQ], DT_BF)
        ksnT = ptile("ksnT", [128, 2 * NS], DT_BF)
        vsn = ptile("vsn", [NS, HD], DT_BF)
        attns = ptile("attns", [128, 2 * H4 * NS], DT_BF)  # [qc | h | s]
        ctx_sT = ptile("ctx_sT", [128, 2 * LQ], DT_BF)     # [hd-c | q]
        ctx_wT = ptile("ctx_wT", [128, 2 * LQ], DT_BF)
        out_w0 = ptile("out_w0", [128, HD], DT_F32)        # [q, h*64]
        out_w1 = ptile("out_w1", [128, HD], DT_F32)
        out_ws = (out_w0, out_w1)

        # ================= phase 1: projections =================
        with tc.tile_pool(name="pj_ps", bufs=1, space="PSUM") as pjp:
            def proj_T(wsb, xsb, ncols, dst, pbias):
                # dst[hd-chunk, :] = w.T @ xT + b   (per-partition bias)
                for hdc in range(2):
                    ps = pjp.tile([128, ncols], DT_F32, name="pjt",
                                  tag="pjt", bufs=2, padded_shape=[128, 512])
                    for dc in range(4):
                        nc.tensor.matmul(
                            ps[:],
                            wsb[:, dc * HD + hdc * 128: dc * HD + hdc * 128 + 128],
                            xsb[:, dc * ncols:(dc + 1) * ncols],
                            start=(dc == 0), stop=(dc == 3))
                    nc.scalar.activation(
                        dst[:, hdc * ncols:(hdc + 1) * ncols], ps[:],
                        AF.Identity, bias=sb_pb[pbias][:, hdc:hdc + 1])

            proj_T(sb_wq_s, sb_qT, LQ, qsT, "pb_q_s")
            proj_T(sb_wq_w, sb_qT, LQ, qwT, "pb_q_w")
            proj_T(sb_wk_s, sb_k_sT, NS, ksnT, "pb_k_s")

            # vsn [24, HD] = v_s @ wv_s + bv_s
            ps_vsn = pjp.tile([NS, HD], DT_F32, name="ps_vsn", tag="vsn", bufs=1)
            for dc in range(4):
                nc.tensor.matmul(ps_vsn[:], sb_v_sT[:, dc * NS:(dc + 1) * NS],
                                 sb_wv_s[:, dc * HD:(dc + 1) * HD],
                                 start=(dc == 0), stop=False)
            nc.tensor.matmul(ps_vsn[:], ones_row[:, :NS], sb_rb_v_s[:],
                             start=False, stop=True)
            nc.scalar.copy(vsn[:], ps_vsn[:])

        with tc.tile_pool(name="kw_ps", bufs=1, space="PSUM") as kwp, \
             tc.tile_pool(name="kw_in", bufs=3) as kwin:
            for stc in range(6):     # kwnT in 512-col chunks
                rhs = kwin.tile([128, 4 * 512], DT_BF, name="kw_rhs", tag="kw_rhs")
                nc.sync.dma_start(
                    rhs[:].rearrange("p (c n) -> p c n", c=4),
                    k_wT.ap()[:, stc * 512:(stc + 1) * 512]
                        .rearrange("(c p) n -> p c n", p=128))
                for hdc in range(2):
                    ps = kwp.tile([128, 512], DT_F32, name="kw_ps_t",
                                  tag="kwt", bufs=2)
                    for dc in range(4):
                        nc.tensor.matmul(
                            ps[:],
                            sb_wk_w[:, dc * HD + hdc * 128: dc * HD + hdc * 128 + 128],
                            rhs[:, dc * 512:(dc + 1) * 512],
                            start=(dc == 0), stop=(dc == 3))
                    nc.scalar.activation(
                        kwnT[:, hdc * (NS * NT) + stc * 512:
                             hdc * (NS * NT) + (stc + 1) * 512], ps[:],
                        AF.Identity, bias=sb_pb["pb_k_w"][:, hdc:hdc + 1])

            for stc in range(6):     # vwn in 4-st-tile chunks
                rhs_v = kwin.tile([128, 4 * 512], DT_BF, name="vw_lhs", tag="vw_lhs")
                nc.sync.dma_start(
                    rhs_v[:].rearrange("p (c n) -> p c n", c=4),
                    v_wT.ap()[:, stc * 512:(stc + 1) * 512]
                        .rearrange("(c p) n -> p c n", p=128))
                for sub in range(4):
                    st = stc * 4 + sub
                    ps = kwp.tile([128, HD], DT_F32, name="vw_ps_t",
                                  tag="vwt", bufs=2, padded_shape=[128, 512])
                    for dc in range(4):
                        nc.tensor.matmul(
                            ps[:], rhs_v[:, dc * 512 + sub * 128: dc * 512 + sub * 128 + 128],
                            sb_wv_w[:, dc * HD:(dc + 1) * HD],
                            start=(dc == 0), stop=False)
                    nc.tensor.matmul(ps[:], ones_row[:], sb_rb_v_w[:],
                                     start=False, stop=True)
                    nc.vector.tensor_copy(
                        vwone[:, st * H4 * UCOL:(st + 1) * H4 * UCOL]
                        .rearrange("p (h u) -> p h u", u=UCOL)[:, :, 0:DK],
                        ps[:].rearrange("p (h d) -> p h d", d=DK))

        # ================= phase 2: sentence branch =================
        with tc.tile_pool(name="sn_ps", bufs=1, space="PSUM") as snp, \
             tc.tile_pool(name="sn_sb", bufs=2) as snb:
            for h in range(H4):
                hdc, off = divmod(h * DK, 128)
                attT = snb.tile([NS, LQ], DT_BF, name="attT", tag="attT")
                for qc in range(2):
                    at0 = snp.tile([128, NS], DT_F32, name="at0", tag="at0", bufs=2)
                    nc.tensor.matmul(
                        at0[:], identB[:],
                        sb_bias_sq[:, (h * 2 + qc) * NS:(h * 2 + qc + 1) * NS],
                        start=True, stop=False)
                    nc.tensor.matmul(
                        at0[:],
                        qsT[off:off + DK, hdc * LQ + qc * 128: hdc * LQ + qc * 128 + 128],
                        ksnT[off:off + DK, hdc * NS:(hdc + 1) * NS],
                        start=False, stop=True)
                    A1u = snb.tile([128, NS], DT_BF, name="A1u", tag="A1u")
                    Z1 = snb.tile([128, 1], DT_F32, name="Z1", tag="Z1")
                    nc.scalar.activation(A1u[:], at0[:], AF.Exp, accum_out=Z1[:])
                    invZ1 = snb.tile([128, 1], DT_F32, name="invZ1", tag="invZ1")
                    nc.vector.reciprocal(invZ1[:], Z1[:])
                    psT = snp.tile([NS, 128], DT_BF, name="psT", tag="psT", bufs=1,
                                   padded_shape=[128, 128])
                    nc.tensor.transpose(psT[:], A1u[:], identB[:])
                    A1uT = snb.tile([NS, 128], DT_BF, name="A1uT", tag="A1uT")
                    nc.vector.tensor_copy(A1uT[:], psT[:])
                    g_u = snp.tile([128, NS], DT_F32, name="g_u", tag="g_u", bufs=1)
                    nc.tensor.matmul(g_u[:], A1uT[:], sb_G[:, h * NS:(h + 1) * NS],
                                     start=True, stop=True)
                    sq = snb.tile([128, NS], DT_F32, name="sq", tag="sq")
                    nc.scalar.activation(sq[:], g_u[:], AF.Square, scale=invZ1[:])
                    att2 = snb.tile([128, NS], DT_F32, name="att2", tag="att2")
                    nc.vector.scalar_tensor_tensor(
                        att2[:], sq[:], -0.5, at0[:], ALU.mult, ALU.add)
                    A2u = snb.tile([128, NS], DT_BF, name="A2u", tag="A2u")
                    Z2 = snb.tile([128, 1], DT_F32, name="Z2", tag="Z2")
                    nc.scalar.activation(A2u[:], att2[:], AF.Exp, accum_out=Z2[:])
                    invZ2 = snb.tile([128, 1], DT_F32, name="invZ2", tag="invZ2")
                    nc.vector.reciprocal(invZ2[:], Z2[:])
                    nc.vector.tensor_scalar_mul(
                        attns[:, (qc * H4 + h) * NS:(qc * H4 + h + 1) * NS],
                        A2u[:], invZ2[:])
                    psT2 = snp.tile([NS, 128], DT_BF, name="psT2", tag="psT2",
                                    bufs=1, padded_shape=[128, 128])
                    nc.tensor.transpose(
                        psT2[:],
                        attns[:, (qc * H4 + h) * NS:(qc * H4 + h + 1) * NS],
                        identB[:])
                    nc.vector.tensor_copy(attT[:, qc * 128:(qc + 1) * 128], psT2[:])
                ps_ctx = snp.tile([DK, LQ], DT_F32, name="ps_ctx", tag="ctx", bufs=1)
                nc.tensor.matmul(ps_ctx[:], vsn[:, h * DK:(h + 1) * DK], attT[:],
                                 start=True, stop=True)
                nc.scalar.copy(
                    ctx_sT[off:off + DK, hdc * LQ:(hdc + 1) * LQ], ps_ctx[:])

        # ================= phase 3: word branch =================
        with tc.tile_pool(name="wd_sc", bufs=1, space="PSUM") as wsc, \
             tc.tile_pool(name="wd_u", bufs=1, space="PSUM") as wup, \
             tc.tile_pool(name="wd_sb", bufs=2) as wsb_pool:
            for h in range(H4):
                hdc, off = divmod(h * DK, 128)
                bt = wsb_pool.tile([128, NS * LQ], DT_BF, name="bt", tag="bt")
                nc.sync.dma_start(
                    bt[:].rearrange("t (s q) -> t s q", s=NS),
                    biasT.ap()[:, h].rearrange("s t q -> t s q"))
                probs = wsb_pool.tile([128, NS * LQ], DT_BF, name="probs", tag="probs")
                for grp in range(6):          # 4 sentences / group (2 banks)
                    sc = wsc.tile([128, 4 * LQ], DT_F32, name="sc", tag="sc", bufs=2)
                    for bank in range(2):         # keep each bank's group contiguous
                        for i2 in range(2):
                            i = bank * 2 + i2
                            s = grp * 4 + i
                            nc.tensor.matmul(sc[:, i * LQ:(i + 1) * LQ], identB[:],
                                             bt[:, s * LQ:(s + 1) * LQ],
                                             start=(i2 == 0), stop=False)
                        for i2 in range(2):
                            i = bank * 2 + i2
                            s = grp * 4 + i
                            nc.tensor.matmul(
                                sc[:, i * LQ:(i + 1) * LQ],
                                kwnT[off:off + DK,
                                     hdc * NS * NT + s * NT: hdc * NS * NT + (s + 1) * NT],
                                qwT[off:off + DK, hdc * LQ:(hdc + 1) * LQ],
                                start=False, stop=(i2 == 1))
                    nc.scalar.activation(
                        probs[:, grp * 4 * LQ:(grp + 1) * 4 * LQ], sc[:], AF.Exp)
                for qc in range(2):
                    for half in range(2):
                        U = wup.tile([128, 2 * 512], DT_F32, name="U", tag="U", bufs=2)
                        for j in range(S_HALF):
                            s = half * S_HALF + j
                            col = 512 * (j // S_PER_BANK) + UCOL * (j % S_PER_BANK)
                            nc.tensor.matmul(
                                U[:, col:col + UCOL],
                                probs[:, s * LQ + qc * 128: s * LQ + qc * 128 + 128],
                                vwone[:, (s * H4 + h) * UCOL:(s * H4 + h + 1) * UCOL],
                                start=True, stop=True)
                        U4 = U[:].rearrange("p (b u) -> p b u", b=2)[:, :, :S_PER_BANK * UCOL] \
                            .rearrange("p b (j u) -> p b j u", u=UCOL)
                        invZ = wsb_pool.tile([128, S_HALF], DT_F32,
                                             name="invZ", tag="invZ")
                        nc.vector.reciprocal(
                            invZ[:].rearrange("p (b j) -> p b j", b=2),
                            U4[:, :, :, DK:DK + 1].squeeze(-1))
                        cvec = wsb_pool.tile([128, S_HALF], DT_F32,
                                             name="cvec", tag="cvec")
                        nc.vector.tensor_mul(
                            cvec[:], invZ[:],
                            attns[:, (qc * H4 + h) * NS + half * S_HALF:
                                  (qc * H4 + h) * NS + half * S_HALF + S_HALF])
                        tmp = wsb_pool.tile([128, S_HALF * DK], DT_F32,
                                            name="tmpw", tag="tmpw")
                        nc.vector.tensor_tensor(
                            tmp[:].rearrange("p (b j d) -> p b j d", b=2, j=S_PER_BANK),
                            U4[:, :, :, 0:DK],
                            cvec[:].rearrange("p (b j) -> p b j", b=2)
                                .unsqueeze(-1).to_broadcast((128, 2, S_PER_BANK, DK)),
                            ALU.mult)
                        if half == 0:
                            nc.vector.reduce_sum(
                                out_ws[qc][:, h * DK:(h + 1) * DK],
                                tmp[:].rearrange("p (s d) -> p d s", d=DK),
                                axis=mybir.AxisListType.X)
                        else:
                            red1 = wsb_pool.tile([128, DK], DT_F32,
                                                 name="red1", tag="red1")
                            nc.vector.reduce_sum(
                                red1[:],
                                tmp[:].rearrange("p (s d) -> p d s", d=DK),
                                axis=mybir.AxisListType.X)
                            nc.vector.tensor_add(
                                out_ws[qc][:, h * DK:(h + 1) * DK],
                                out_ws[qc][:, h * DK:(h + 1) * DK], red1[:])

        # ================= phase 4: transpose out_w + final =================
        with tc.tile_pool(name="fn_ps", bufs=1, space="PSUM") as fnp, \
             tc.tile_pool(name="fn_sb", bufs=1) as fnb:
            for qc in range(2):
                for hdc in range(2):
                    pst = fnp.tile([128, 128], DT_F32, name="pstw", tag="pstw",
                                   bufs=2)
                    nc.tensor.transpose(
                        pst[:], out_ws[qc][:, hdc * 128:(hdc + 1) * 128], identF[:])
                    nc.scalar.copy(
                        ctx_wT[:, hdc * LQ + qc * 128: hdc * LQ + qc * 128 + 128],
                        pst[:])
            for qc in range(2):
                ps = fnp.tile([128, D], DT_F32, name="fn", tag="fn", bufs=2)
                for hdc in range(2):
                    nc.tensor.matmul(
                        ps[:],
                        ctx_sT[:, hdc * LQ + qc * 128: hdc * LQ + qc * 128 + 128],
                        sb_Ws[:, hdc * D:(hdc + 1) * D],
                        start=(hdc == 0), stop=False)
                    nc.tensor.matmul(
                        ps[:],
                        ctx_wT[:, hdc * LQ + qc * 128: hdc * LQ + qc * 128 + 128],
                        sb_Ww[:, hdc * D:(hdc + 1) * D],
                        start=False, stop=(hdc == 1))
                o = fnb.tile([128, D], DT_F32, name="o_sb", tag="o_sb", bufs=2)
                nc.vector.tensor_copy(o[:], ps[:])
                nc.sync.dma_start(out_t.ap()[qc * 128:(qc + 1) * 128, :], o[:])

    return dram, out_t


def shard_inputs(inputs):
    f32 = {k: np.asarray(v, np.float32) for k, v in inputs.items()}
    fco = f32['fco_w']
    ident = np.eye(128, dtype=np.float32)
    in_maps = []
    for c in range(N_CORES):
        b, g = divmod(c, 2)
        hsl = slice(g * HD, (g + 1) * HD)
        heads = slice(g * H4, (g + 1) * H4)
        m = {}
        m['qT'] = np.ascontiguousarray(f32['q'][b].T).astype(BF16)
        m['k_sT'] = np.ascontiguousarray(f32['k_s'][b].T).astype(BF16)
        m['v_sT'] = np.ascontiguousarray(f32['v_s'][b].T).astype(BF16)
        m['k_wT'] = np.ascontiguousarray(
            f32['k_w'][b].reshape(NS * NT, D).T).astype(BF16)
        m['v_wT'] = np.ascontiguousarray(
            f32['v_w'][b].reshape(NS * NT, D).T).astype(BF16)
        m['biasT'] = np.ascontiguousarray(
            f32['bias_w'][b][:, heads].transpose(0, 1, 3, 2)).astype(BF16)
        m['bias_sq'] = np.ascontiguousarray(f32['bias_s'][b][heads]).astype(BF16)
        m['G'] = np.ascontiguousarray(f32['graph_attn_bias'][b][heads]).astype(BF16)
        m['wq_s'] = (f32['wq_s'][:, hsl] * SCALE).astype(BF16)
        m['wk_s'] = f32['wk_s'][:, hsl].astype(BF16)
        m['wv_s'] = f32['wv_s'][:, hsl].astype(BF16)
        m['wq_w'] = (f32['wq_w'][:, hsl] * SCALE).astype(BF16)
        m['wk_w'] = f32['wk_w'][:, hsl].astype(BF16)
        m['wv_w'] = f32['wv_w'][:, hsl].astype(BF16)
        m['Ws_eff'] = (f32['fcs_w'][hsl] @ fco[:D]).astype(BF16)
        m['Ww_eff'] = (f32['fcw_w'][hsl] @ fco[D:]).astype(BF16)
        m['ident_bf'] = ident.astype(BF16)
        m['ident_f32'] = ident.copy()
        m['pb_q_s'] = np.ascontiguousarray(
            (f32['bq_s'][hsl] * SCALE).reshape(2, 128).T)
        m['pb_k_s'] = np.ascontiguousarray(f32['bk_s'][hsl].reshape(2, 128).T)
        m['pb_q_w'] = np.ascontiguousarray(
            (f32['bq_w'][hsl] * SCALE).reshape(2, 128).T)
        m['pb_k_w'] = np.ascontiguousarray(f32['bk_w'][hsl].reshape(2, 128).T)
        m['rb_v_s'] = f32['bv_s'][hsl].reshape(1, HD).astype(BF16)
        m['rb_v_w'] = f32['bv_w'][hsl].reshape(1, HD).astype(BF16)
        in_maps.append(m)
    return in_maps


def gather_outputs(results, inputs):
    f32 = {k: np.asarray(v, np.float32)
           for k, v in inputs.items()
           if k in ('fcs_b', 'fcw_b', 'fco_b', 'fco_w')}
    fco = f32['fco_w']
    v0 = f32['fcs_b'] @ fco[:D] + f32['fcw_b'] @ fco[D:] + f32['fco_b']
    out = np.zeros((B, LQ, D), np.float32)
    for b in range(B):
        out[b] = results[2 * b]['out'] + results[2 * b + 1]['out'] + v0
    return out


_CACHE = {}


def _get_nc():
    if 'nc' not in _CACHE:
        from concourse import bacc
        nc = bacc.Bacc("TRN2", target_bir_lowering=False, debug=False)
        build_core_program(nc)
        nc.compile()
        _CACHE['nc'] = nc
    return _CACHE['nc']


def kernel(**inputs):
    from concourse import bass_utils
    nc = _get_nc()
    in_maps = shard_inputs(inputs)
    res = bass_utils.run_bass_kernel_spmd(
        nc, in_maps, core_ids=list(range(N_CORES)))
    return gather_outputs(res.results, inputs)


if __name__ == "__main__":
    # trace-only smoke test: build the program and print stats
    nc = _get_nc()
    n_inst = sum(len(bb.instructions) for f in nc.m.functions
                 for bb in f.basicblocks) if hasattr(nc.m.functions[0], 'basicblocks') else -1
    print("build OK")
